# revision 1
# baseline (speedup 1.0000x reference)
"""GemLite int4-quantized linear: out = x @ dequant(W_q, scales, zeros).

Column-parallel across 8 NeuronCores: W_q/scales/zeros sharded along
out_features (N), x replicated, outputs concatenated.

Shapes (hardcoded from the problem spec):
  x      [128, 8192] f32
  W_q    [1024, 8192] int32   (each int32 packs 8 x 4-bit along K, LSB first)
  scales [64, 8192] f32       (group_size=128 along K)
  zeros  [64, 8192] f32
  out    [128, 8192] f32
"""

import numpy as np

M = 128
K = 8192
N = 8192
GROUP_SIZE = 128
NBITS = 4
EPS = 8  # elems per int32 sample
NCORES = 8
N_LOC = N // NCORES

_JAX_FN = None


def _build_jax_fn():
    global _JAX_FN
    if _JAX_FN is not None:
        return _JAX_FN
    import jax
    import jax.numpy as jnp
    from jax.sharding import Mesh, NamedSharding, PartitionSpec as P

    devs = jax.devices()[:NCORES]
    mesh = Mesh(np.array(devs), ("x",))

    def shard_fn(x, W_q, scales, zeros):
        # per-shard dequant + matmul; all arrays already sharded on N
        shifts = jnp.arange(EPS, dtype=jnp.int32) * NBITS
        u = (W_q[:, None, :] >> shifts[None, :, None]) & 15
        u = u.reshape(K, N_LOC).astype(jnp.float32)
        s = jnp.repeat(scales, GROUP_SIZE, axis=0)
        z = jnp.repeat(zeros, GROUP_SIZE, axis=0)
        return jnp.matmul(x, (u - z) * s, preferred_element_type=jnp.float32)

    from jax.experimental.shard_map import shard_map

    fn = shard_map(
        shard_fn,
        mesh=mesh,
        in_specs=(P(), P(None, "x"), P(None, "x"), P(None, "x")),
        out_specs=P(None, "x"),
    )
    _JAX_FN = jax.jit(fn)
    return _JAX_FN


def kernel(x, W_q, scales, zeros):
    fn = _build_jax_fn()
    out = fn(
        np.asarray(x, dtype=np.float32),
        np.asarray(W_q, dtype=np.int32),
        np.asarray(scales, dtype=np.float32),
        np.asarray(zeros, dtype=np.float32),
    )
    return np.asarray(out, dtype=np.float32)
